# revision 2
# baseline (speedup 1.0000x reference)
"""BFP-quantized linear kernel for Trainium2, 8-core SPMD — v2.

out = bfp_quantize(input) @ bfp_quantize(weight).T + bias
  input  [8192, 4608] f32, weight [4608, 4608] f32, bias [4608] f32
  BFP: groups of 36 contiguous elements (along rows), shared exponent
  from the group absmax, mantissas truncated toward zero to 8 bits.

v2 changes vs v1 (1310 us):
  * quant via fmod: q = g - fmod(g, step) with step = 2^(e-7) built by
    bit arithmetic on the group absmax: step_bits =
    (absmax_bits & 0x7F800000) - (7 << 23).  fmod (C-style, trunc) by a
    power of two is exact, handles |g| < step (-> 0) and all-zero
    groups (-> 0) with no extra masking.  2 full-size elementwise ops
    (mod on GpSimd, subtract on DVE) instead of 6 DVE + 1 GpSimd.
  * weight-first emission: all weight quant/transpose/bounce before any
    input work, so the 4 quarter-AllGathers fire back-to-back early.
  * main loop pairs the two 512-col n-halves per (ob, kt) so the lhsT
    weight tile is loaded once per pair (LDWEIGHTS halved).
"""

import numpy as np

import concourse.bass as bass
import concourse.mybir as mybir
import concourse.tile as tile
from concourse import bacc
from concourse import bass_utils
from concourse.masks import make_identity

N_CORES = 8
N_ROWS, K_IN, O_OUT = 8192, 4608, 4608
NSH = N_ROWS // N_CORES   # 1024 input rows per core
OSH = O_OUT // N_CORES    # 576 weight rows per core
GS = 36                   # BFP group size
KT = K_IN // 128          # 36 k tiles
NB = NSH // 128           # 8 n blocks per core
OB_TOT = O_OUT // 128     # 36 o blocks
CHUNK = 1152              # quantization column chunk (32 groups)
NG = CHUNK // GS          # 32 groups per chunk

F32 = mybir.dt.float32
BF16 = mybir.dt.bfloat16
I32 = mybir.dt.int32
I16 = mybir.dt.int16


def _emit_quant2(nc, tpool, src, qdst, rows, c0):
    """Quantize src[:rows, c0:c0+CHUNK] (f32) into qdst[:rows, c0:c0+CHUNK]
    (bf16). 16-bit-domain recipe (probe-validated exact on HW):

      q only needs the top 16 bits of x (q is bf16-exact), so all full-size
      ALU runs on packed int16 at 2x/4x DVE rate:
        xh  = high halves of x            (Scalar strided copy)
        e7  = amax_hi & 0x7F80            (exponent<<7 of group absmax)
        exd = xh & 0x7F80
        d7  = e7 - exd                    (scalar_tensor_tensor)
        s   = d7 >> 7;  zm = (d7 < 1024) as bf16
        q   = bf16((xh >> s) << s) * zm   (shift count unclamped: zm zeroes
                                           every d>=8 lane and the masked
                                           bits can never form a NaN)
    """
    xs = src[:rows, c0 : c0 + CHUNK]

    absmax = tpool.tile([128, NG], F32, tag="absmax", name="absmax")
    nc.vector.tensor_reduce(
        out=absmax[:rows], in_=xs.rearrange("p (g e) -> p g e", e=GS),
        axis=mybir.AxisListType.X,
        op=mybir.AluOpType.max, apply_absolute_value=True,
    )
    e_b7 = tpool.tile([128, NG], I16, tag="e_b7", name="e_b7")
    amax_hi = absmax[:rows].bitcast(I16).rearrange("p (g t) -> p g t", t=2)[:, :, 1]
    nc.vector.tensor_scalar(
        out=e_b7[:rows], in0=amax_hi, scalar1=0x7F80, scalar2=None,
        op0=mybir.AluOpType.bitwise_and,
    )
    xh = tpool.tile([128, CHUNK], I16, tag="xh", name="xh")
    xpairs = xs.bitcast(BF16).rearrange("p (k t) -> p k t", t=2)
    nc.scalar.copy(xh[:rows].bitcast(BF16), xpairs[:, :, 1])

    d7 = tpool.tile([128, CHUNK], I16, tag="d7", name="d7")
    nc.vector.tensor_scalar(
        out=d7[:rows], in0=xh[:rows], scalar1=0x7F80, scalar2=None,
        op0=mybir.AluOpType.bitwise_and,
    )
    nc.vector.scalar_tensor_tensor(
        out=d7[:rows].rearrange("p (g e) -> p g e", e=GS),
        in0=d7[:rows].rearrange("p (g e) -> p g e", e=GS),
        scalar=-1,
        in1=e_b7[:rows].unsqueeze(-1).broadcast_to([rows, NG, GS]),
        op0=mybir.AluOpType.mult,
        op1=mybir.AluOpType.add,
    )
    zm = tpool.tile([128, CHUNK], BF16, tag="zm", name="zm")
    nc.vector.tensor_scalar(
        out=zm[:rows], in0=d7[:rows], scalar1=1024, scalar2=None,
        op0=mybir.AluOpType.is_lt,
    )
    nc.vector.tensor_scalar(
        out=d7[:rows], in0=d7[:rows], scalar1=7, scalar2=None,
        op0=mybir.AluOpType.logical_shift_right,
    )
    nc.vector.tensor_tensor(
        out=xh[:rows], in0=xh[:rows], in1=d7[:rows],
        op=mybir.AluOpType.logical_shift_right,
    )
    nc.vector.tensor_tensor(
        out=xh[:rows], in0=xh[:rows], in1=d7[:rows],
        op=mybir.AluOpType.logical_shift_left,
    )
    nc.gpsimd.tensor_tensor(
        out=qdst[:rows, c0 : c0 + CHUNK], in0=xh[:rows].bitcast(BF16),
        in1=zm[:rows], op=mybir.AluOpType.mult,
    )


def emit_kernel(tc, nc, x_d, w_d, b_d, o_d):
    HALF = K_IN // 2
    NQ = 4
    QW = K_IN // NQ  # 1152 k per quarter = 9 k-tiles

    with (
        tc.tile_pool(name="dram", bufs=1, space="DRAM") as dpool,
        tc.tile_pool(name="consts", bufs=1) as cpool,
        tc.tile_pool(name="stage", bufs=2) as spool,
        tc.tile_pool(name="qtmps", bufs=2) as tpool,
        tc.tile_pool(name="qnat", bufs=2) as qpool,
        tc.tile_pool(name="qxt", bufs=1) as xtpool,
        tc.tile_pool(name="wstream", bufs=38) as wpool,
        tc.tile_pool(name="tstage", bufs=4) as tspool,
        tc.tile_pool(name="outs", bufs=4) as opool,
        tc.tile_pool(name="pmm", bufs=4, space="PSUM") as pmm,
        tc.tile_pool(name="ptp", bufs=3, space="PSUM") as ptp,
    ):
        ident = cpool.tile([128, 128], BF16, name="ident")
        make_identity(nc, ident[:])
        # biasT[p, ob] = bias[ob*128 + p]
        biasT = cpool.tile([128, OB_TOT], F32, name="biasT")
        nc.sync.dma_start(
            out=biasT[:], in_=b_d.rearrange("(o p) -> p o", p=128)
        )

        # ---------- weight shard: quantize + transpose + bounce ----------
        # All weight work first so the AllGather chain starts early and the
        # quarters pipeline tightly.
        w_tiles = [(i * 128, min(128, OSH - i * 128)) for i in range((OSH + 127) // 128)]
        qw_boun = [
            dpool.tile([QW, OSH], BF16, name=f"qw_boun{q}") for q in range(NQ)
        ]
        qwt_g = [
            dpool.tile(
                [N_CORES * QW, OSH], BF16, addr_space="Shared", name=f"qwt_g{q}"
            )
            for q in range(NQ)
        ]
        for q in range(NQ):
            for r0, rows in w_tiles:
                wtile = spool.tile([128, QW], F32, tag="stage", name="wtile")
                nc.sync.dma_start(
                    out=wtile[:rows], in_=w_d[r0 : r0 + rows, q * QW : (q + 1) * QW]
                )
                qw = qpool.tile([128, QW], BF16, tag="qnat", name="qw")
                _emit_quant2(nc, tpool, wtile, qw, rows, 0)
                for ktl in range(QW // 128):
                    pt = ptp.tile([128, 128], BF16, tag="tp", name="pt")
                    nc.tensor.transpose(
                        pt[:, :rows], qw[:rows, ktl * 128 : (ktl + 1) * 128],
                        ident[:rows, :rows],
                    )
                    st = tspool.tile([128, 128], BF16, tag="ts", name="st")
                    nc.scalar.copy(st[:, :rows], pt[:, :rows])
                    nc.sync.dma_start(
                        out=qw_boun[q][ktl * 128 : (ktl + 1) * 128, r0 : r0 + rows],
                        in_=st[:, :rows],
                    )
            nc.gpsimd.collective_compute(
                "AllGather",
                mybir.AluOpType.bypass,
                replica_groups=[list(range(N_CORES))],
                ins=[qw_boun[q][:].opt()],
                outs=[qwt_g[q][:].opt()],
            )

        # ---------- input shard: quantize + PE transpose into resident qxT ----------
        qxT = [
            xtpool.tile([128, NSH], BF16, name=f"qxT{kt}") for kt in range(KT)
        ]
        for nb in range(NB):
            for h in range(2):
                k0 = h * HALF
                xtile = spool.tile([128, HALF], F32, tag="stage", name="xtile")
                nc.sync.dma_start(
                    out=xtile[:], in_=x_d[nb * 128 : (nb + 1) * 128, k0 : k0 + HALF]
                )
                qx = qpool.tile([128, HALF], BF16, tag="qnat", name="qx")
                for ch in range(HALF // CHUNK):
                    _emit_quant2(nc, tpool, xtile, qx, 128, ch * CHUNK)
                for ktl in range(KT // 2):
                    kt = h * (KT // 2) + ktl
                    pt = ptp.tile([128, 128], BF16, tag="tp", name="pt")
                    nc.tensor.transpose(
                        pt[:], qx[:, ktl * 128 : (ktl + 1) * 128], ident[:]
                    )
                    nc.scalar.copy(qxT[kt][:, nb * 128 : (nb + 1) * 128], pt[:])

        # ---------- matmul: per (ob) a pair of psum [o128, n512] halves ----------
        # gathered layout: quarter q holds shard c at rows [c*QW, (c+1)*QW)
        for og in range(4):  # o-group = 1152 cols = shards 2og, 2og+1
            wq = []
            for kt in range(KT):
                q, ktl = kt // (QW // 128), kt % (QW // 128)
                wqt = wpool.tile([128, 2 * OSH], BF16, tag="wq", name="wqt")
                for h in range(2):
                    c = 2 * og + h
                    nc.sync.dma_start(
                        out=wqt[:, h * OSH : (h + 1) * OSH],
                        in_=qwt_g[q][c * QW + ktl * 128 : c * QW + (ktl + 1) * 128, :],
                    )
                wq.append(wqt)
            for obl in range(9):
                ob = og * 9 + obl
                psA = pmm.tile([128, 512], F32, tag="mm", name="psA")
                psB = pmm.tile([128, 512], F32, tag="mm", name="psB")
                for kt in range(KT):
                    lhsT = wq[kt][:, obl * 128 : (obl + 1) * 128]
                    nc.tensor.matmul(
                        psA[:], lhsT, qxT[kt][:, 0:512],
                        start=(kt == 0), stop=(kt == KT - 1),
                    )
                    nc.tensor.matmul(
                        psB[:], lhsT, qxT[kt][:, 512:1024],
                        start=(kt == 0), stop=(kt == KT - 1),
                    )
                for h, ps in ((0, psA), (1, psB)):
                    ot = opool.tile([128, 512], F32, tag="ot", name="ot")
                    nc.scalar.activation(
                        ot[:], ps[:],
                        mybir.ActivationFunctionType.Identity,
                        bias=biasT[:, ob : ob + 1], scale=1.0,
                    )
                    nc.sync.dma_start(
                        out=o_d[ob * 128 : (ob + 1) * 128, h * 512 : (h + 1) * 512],
                        in_=ot[:],
                    )


_CACHED_NC = None


def _build():
    global _CACHED_NC
    if _CACHED_NC is not None:
        return _CACHED_NC
    nc = bacc.Bacc(
        "TRN2", target_bir_lowering=False, debug=False, num_devices=N_CORES
    )
    x_d = nc.dram_tensor("x", [NSH, K_IN], F32, kind="ExternalInput").ap()
    w_d = nc.dram_tensor("w", [OSH, K_IN], F32, kind="ExternalInput").ap()
    b_d = nc.dram_tensor("b", [O_OUT], F32, kind="ExternalInput").ap()
    o_d = nc.dram_tensor("o", [O_OUT, NSH], F32, kind="ExternalOutput").ap()
    with tile.TileContext(nc) as tc:
        emit_kernel(tc, nc, x_d, w_d, b_d, o_d)
    nc.compile()
    _CACHED_NC = nc
    return nc


def _ensure_axon_hooks_importable():
    import sys
    import types

    if "antenv.axon_hooks" not in sys.modules:
        try:
            import antenv.axon_hooks  # noqa: F401
        except ImportError:
            mod = types.ModuleType("antenv.axon_hooks")
            mod.get_axon_ntff_profile_hook = lambda: None
            mod.set_axon_ntff_profile_hook = lambda h: None
            sys.modules["antenv.axon_hooks"] = mod


def run_on_hw(input, weight, bias, trace=False):
    _ensure_axon_hooks_importable()
    nc = _build()
    in_maps = []
    for c in range(N_CORES):
        in_maps.append(
            {
                "x": np.ascontiguousarray(input[c * NSH : (c + 1) * NSH]),
                "w": np.ascontiguousarray(weight[c * OSH : (c + 1) * OSH]),
                "b": np.ascontiguousarray(bias),
            }
        )
    res = bass_utils.run_bass_kernel_spmd(
        nc, in_maps, core_ids=list(range(N_CORES)), trace=trace
    )
    out = np.empty((N_ROWS, O_OUT), dtype=np.float32)
    for c in range(N_CORES):
        out[c * NSH : (c + 1) * NSH] = res.results[c]["o"].T
    return out, res


def kernel(input, weight, bias):
    out, _ = run_on_hw(
        np.asarray(input, dtype=np.float32),
        np.asarray(weight, dtype=np.float32),
        np.asarray(bias, dtype=np.float32),
    )
    return out
